# revision 1
# baseline (speedup 1.0000x reference)
"""Trainium2 Bass kernel for nn_CFModule_12575664243188.

Module (per batch b of x[B, H, W, C]):
  pooled = AdaptiveAvgPool2d((4,4))(x)            # [4, 4, C] window means
  xf     = pooled.reshape(16, C).T                # [C, 16]
  dots   = (xf @ xf.T) * 16**-0.5                 # [C, C]
  attn   = softmax(dots, axis=-1)
  out    = einsum('hwc,tc->hwt', x, attn)
  y      = gelu(out, exact erf)

Sharding: pure data-parallel, B=16 over 8 cores (2 batches/core).

Device layout trick: the host pre-transposes each batch to
xt[b, g*64+c, f] = x[b, g*HWH + f, c]  (g = hw half, HWH = H*W/2), so the
contraction channel dim c sits on SBUF partitions.  The main einsum is then
a single PE matmul per tile with a block-diagonal stationary matrix
  lhsT2[g*64+c, g'*64+t] = attnT[c, t] * (g == g')
giving out psum[g*64+t, f] which stores contiguously to yt[b, g*64+t, f];
the host transposes back.  Pooling = two DVE free-dim reduces.  GELU runs on
the scalar engine straight out of PSUM (fused with the PSUM->SBUF copy).
"""

from contextlib import ExitStack

import numpy as np

import concourse.bacc as bacc
import concourse.bass as bass
import concourse.mybir as mybir
import concourse.tile as tile
from concourse.bass_utils import run_bass_kernel_spmd

# Problem shapes (hardcoded per spec)
B, H, W, C = 16, 256, 256, 64
N_CORES = 8
NB = B // N_CORES          # batches per core
HW = H * W
G = 2                      # hw halves packed on partitions
HWH = HW // G              # 32768
CH = 4096                  # hw elements per SBUF chunk (per half)
N_CHUNKS = HWH // CH       # 8
MM_N = 512                 # moving free dim per matmul
OSTAGE = 4096              # out staging columns per DMA store
WIN_H, WIN_W = H // 4, W // 4          # 64 x 64 pooling windows
WIN_ELEMS = WIN_H * WIN_W              # 4096
# dots = (pooled_sums / WIN_ELEMS^2) * 16^-0.5, folded into the exp() scale
SCALE_TOTAL = float(16.0 ** -0.5 / (WIN_ELEMS * WIN_ELEMS))

F32 = mybir.dt.float32
AF = mybir.ActivationFunctionType
# CoreSim does not implement Gelu; tests may swap this for AF.Copy.
GELU_FUNC = AF.Gelu
# Main-matmul dtype: float32r streams the PE fmap at 1 cycle/row (vs 4 for
# float32) at the cost of ~2e-4 relative error from fmap/weight rounding.
MM_DT = mybir.dt.float32


def build_kernel(ctx: ExitStack, tc: "tile.TileContext", yt: bass.AP,
                 xt: bass.AP, ident: bass.AP, zeros: bass.AP,
                 repeats: int = 1):
    """Emit the per-core program.

    xt:    [NB, 128, HWH] f32 input  (128 = g*64+c)
    ident: [128, 128] f32 identity (for PE transposes)
    yt:    [NB, 128, HWH] f32 output (128 = g*64+t)
    """
    nc = tc.nc
    assert CH % W == 0
    rows_per_chunk = CH // W          # h rows per chunk (within a half)
    half_rows = H // G                # 128 rows per half
    gh_per_half = half_rows // WIN_H  # 2

    const_pool = ctx.enter_context(tc.tile_pool(name="const", bufs=1))
    x_pool = ctx.enter_context(tc.tile_pool(name="x", bufs=9))
    o_pool = ctx.enter_context(tc.tile_pool(name="o", bufs=3))
    r_pool = ctx.enter_context(tc.tile_pool(name="r", bufs=2))
    sm_pool = ctx.enter_context(tc.tile_pool(name="sm", bufs=2))
    ps_out = ctx.enter_context(tc.tile_pool(name="ps_out", bufs=4, space="PSUM"))
    ps_sm = ctx.enter_context(tc.tile_pool(name="ps_sm", bufs=1, space="PSUM"))

    ident_sb = const_pool.tile([128, 128], F32)
    nc.sync.dma_start(ident_sb[:], ident)
    # Per-batch-slot block-diag stationary matrices, zeroed once; only the
    # two diagonal 64x64 blocks are rewritten each batch.
    lhsT2s = []
    for b in range(NB):
        t = const_pool.tile([128, 128], MM_DT, tag=f"lhsT2_{b}")
        nc.sync.dma_start(t[:], zeros.bitcast(MM_DT))
        lhsT2s.append(t)

    for b in [b for _ in range(repeats) for b in range(NB)]:
        # ---- Phase 1: load chunks, accumulate row-window partial sums ----
        # R1[p, h*4 + gw] = sum_w x[b, hw(g, h, gw, w), c]   (h local to half)
        r1 = r_pool.tile([128, half_rows * 4], F32, tag="r1")
        chunks = []
        for k in range(N_CHUNKS):
            xc = x_pool.tile([128, CH], MM_DT, tag="xc")
            nc.sync.dma_start(xc[:], xt[b, :, k * CH:(k + 1) * CH].bitcast(MM_DT))
            chunks.append(xc)
            nc.vector.reduce_sum(
                r1[:, k * rows_per_chunk * 4:(k + 1) * rows_per_chunk * 4],
                xc[:].bitcast(F32).rearrange("p (h g w) -> p h g w",
                                             h=rows_per_chunk, g=4, w=WIN_W),
                axis=mybir.AxisListType.X,
            )

        # ---- Phase 2: finish pooling, attention matrix ----
        # pooled[p, gh*4+gw] = sum over the window (gh local to half)
        pooled = sm_pool.tile([128, 8], F32, tag="pooled")
        nc.vector.reduce_sum(
            pooled[:],
            r1[:].rearrange("p (gh h g) -> p gh g h", gh=gh_per_half,
                            h=WIN_H, g=4),
            axis=mybir.AxisListType.X,
        )
        # pooledT: [8, 128] = pooled.T via PE transpose
        pt_ps = ps_sm.tile([8, 128], F32, tag="pt_ps")
        nc.tensor.transpose(pt_ps[:], pooled[:], ident_sb[:])
        p8 = sm_pool.tile([8, 128], F32, tag="p8")
        nc.vector.tensor_copy(p8[:], pt_ps[:])

        # dots[c1, c2] = sum over all 16 windows (8 per half, summed in PSUM)
        dots_ps = ps_sm.tile([64, 64], F32, tag="dots_ps")
        nc.tensor.matmul(dots_ps[:], p8[:, 0:64], p8[:, 0:64],
                         start=True, stop=False)
        nc.tensor.matmul(dots_ps[:], p8[:, 64:128], p8[:, 64:128],
                         start=False, stop=True)

        # softmax over rows: attn = exp(s*dots - max)/sum
        mx = sm_pool.tile([64, 1], F32, tag="mx")
        nc.vector.reduce_max(mx[:], dots_ps[:], axis=mybir.AxisListType.X)
        nbias = sm_pool.tile([64, 1], F32, tag="nbias")
        nc.vector.tensor_scalar_mul(nbias[:], mx[:], -SCALE_TOTAL)
        e_sb = sm_pool.tile([64, 64], F32, tag="e_sb")
        ssum = sm_pool.tile([64, 1], F32, tag="ssum")
        nc.scalar.activation(e_sb[:], dots_ps[:], AF.Exp, bias=nbias[:],
                             scale=SCALE_TOTAL, accum_out=ssum[:])
        rcp = sm_pool.tile([64, 1], F32, tag="rcp")
        nc.vector.reciprocal(rcp[:], ssum[:])
        attn = sm_pool.tile([64, 64], F32, tag="attn")
        nc.vector.tensor_scalar_mul(attn[:], e_sb[:], rcp[:])

        # lhsT2 = diag(attnT, attnT) [128, 128] f32r; attnT[c,t] = attn[t,c].
        # The g=1 diagonal block is a partition-shifting SBUF->SBUF DMA copy.
        at_ps = ps_sm.tile([64, 64], F32, tag="at_ps")
        nc.tensor.transpose(at_ps[:], attn[:], ident_sb[0:64, 0:64])
        lhsT2 = lhsT2s[b]
        nc.vector.tensor_copy(lhsT2[0:64, 0:64], at_ps[:])
        nc.sync.dma_start(lhsT2[64:128, 64:128], lhsT2[0:64, 0:64])

        # ---- Phase 3: main matmul + gelu + store ----
        # Two concurrent tile-positioned matmuls: half g uses array
        # rows/cols [64g, 64g+64), contracting c over partitions 64g..64g+63.
        for k in range(N_CHUNKS):
            xc = chunks[k]
            for s in range(CH // OSTAGE):
                ostage = o_pool.tile([128, OSTAGE], F32, tag="ostage")
                for j in range(OSTAGE // MM_N):
                    col = s * OSTAGE + j * MM_N
                    ps = ps_out.tile([128, MM_N], F32, tag="ps")
                    nc.tensor.matmul(ps[:], lhsT2[:],
                                     xc[:, col:col + MM_N],
                                     start=True, stop=True)
                    nc.scalar.activation(ostage[:, j * MM_N:(j + 1) * MM_N],
                                         ps[:], GELU_FUNC)
                nc.scalar.dma_start(
                    yt[b, :, k * CH + s * OSTAGE: k * CH + (s + 1) * OSTAGE],
                    ostage[:])


def build_nc(trn_type: str = "TRN2", repeats: int = 1) -> bass.Bass:
    nc = bacc.Bacc(trn_type, debug=False, target_bir_lowering=False)
    xt = nc.dram_tensor("xt", [NB, 128, HWH], F32, kind="ExternalInput")
    ident = nc.dram_tensor("ident", [128, 128], F32, kind="ExternalInput")
    zeros = nc.dram_tensor("zeros", [128, 128], F32, kind="ExternalInput")
    yt = nc.dram_tensor("yt", [NB, 128, HWH], F32, kind="ExternalOutput")
    with tile.TileContext(nc) as tc:
        with ExitStack() as ctx:
            build_kernel(ctx, tc, yt.ap(), xt.ap(), ident.ap(), zeros.ap(),
                         repeats=repeats)
    nc.compile()
    return nc


def _pack_inputs(x: np.ndarray) -> np.ndarray:
    # x [B, H, W, C] -> xt [B, 128, HWH]; xt[b, g*64+c, f] = x[b, g*HWH+f, c]
    xr = x.reshape(B, G, HWH, C).transpose(0, 1, 3, 2)   # [B, G, C, HWH]
    return np.ascontiguousarray(xr.reshape(B, G * C, HWH), dtype=np.float32)


def _unpack_outputs(yt: np.ndarray) -> np.ndarray:
    # yt [B, 128, HWH] -> y [B, H, W, C]
    yr = yt.reshape(B, G, C, HWH).transpose(0, 1, 3, 2)  # [B, G, HWH, C]
    return np.ascontiguousarray(yr.reshape(B, H, W, C))


_cached = {}


def kernel(x: np.ndarray) -> np.ndarray:
    x = np.asarray(x, dtype=np.float32)
    assert x.shape == (B, H, W, C)
    xt = _pack_inputs(x)
    ident = np.eye(128, dtype=np.float32)

    if "nc" not in _cached:
        _cached["nc"] = build_nc()
    nc = _cached["nc"]

    zeros = np.zeros((128, 128), dtype=np.float32)
    in_maps = [
        {"xt": np.ascontiguousarray(xt[i * NB:(i + 1) * NB]), "ident": ident,
         "zeros": zeros}
        for i in range(N_CORES)
    ]
    res = run_bass_kernel_spmd(nc, in_maps, core_ids=list(range(N_CORES)))
    yt = np.concatenate([r["yt"] for r in res.results], axis=0)
    return _unpack_outputs(yt)


if __name__ == "__main__":
    xs = np.random.default_rng(0).standard_normal((B, H, W, C), dtype=np.float32)
    y = kernel(xs)
    print("ok", y.shape, y.dtype)



# revision 3
# speedup vs baseline: 9.6227x; 9.6227x over previous
"""Trainium2 Bass kernel for nn_CFModule_12575664243188.

Module (per batch b of x[B, H, W, C]):
  pooled = AdaptiveAvgPool2d((4,4))(x)            # [4, 4, C] window means
  xf     = pooled.reshape(16, C).T                # [C, 16]
  dots   = (xf @ xf.T) * 16**-0.5                 # [C, C]
  attn   = softmax(dots, axis=-1)
  out    = einsum('hwc,tc->hwt', x, attn)
  y      = gelu(out, exact erf)

Sharding: pure data-parallel, B=16 over 8 cores (2 batches/core).

Device layout trick: the host pre-transposes each batch to
xt[b, g*64+c, f] = x[b, g*HWH + f, c]  (g = hw half, HWH = H*W/2), so the
contraction channel dim c sits on SBUF partitions.  The main einsum is then
a single PE matmul per tile with a block-diagonal stationary matrix
  lhsT2[g*64+c, g'*64+t] = attnT[c, t] * (g == g')
giving out psum[g*64+t, f] which stores contiguously to yt[b, g*64+t, f];
the host transposes back.  Pooling = two DVE free-dim reduces.  GELU runs on
the scalar engine straight out of PSUM (fused with the PSUM->SBUF copy).

Precision: all HBM I/O and the main matmul run in bf16 (error budget is
2e-2; bf16 rounding contributes ~4e-3 worst-case).  This halves HBM traffic
vs f32 (the roofline bottleneck: ~33.5 MB per core at ~358 GB/s) and
streams the PE fmap at 1 cycle/column instead of 4.  Pooling sums, softmax
and PSUM accumulation stay f32.
"""

from contextlib import ExitStack

import ml_dtypes
import numpy as np

import concourse.bacc as bacc
import concourse.bass as bass
import concourse.mybir as mybir
import concourse.tile as tile
from concourse.bass_utils import run_bass_kernel_spmd

# Problem shapes (hardcoded per spec)
B, H, W, C = 16, 256, 256, 64
N_CORES = 8
NB = B // N_CORES          # batches per core
HW = H * W
G = 2                      # hw halves packed on partitions
HWH = HW // G              # 32768
CH = 16384                 # hw elements per SBUF chunk (per half) -> 4MB DMA
N_CHUNKS = HWH // CH       # 2
MM_N = 512                 # moving free dim per matmul (= 1 PSUM bank f32)
PS_N = 2048                # PSUM tile columns (4 banks) per gelu ACT read
OSTAGE = 8192              # out staging columns per DMA store (2MB bf16)
WIN_H, WIN_W = H // 4, W // 4          # 64 x 64 pooling windows
WIN_ELEMS = WIN_H * WIN_W              # 4096
# dots = (pooled_sums / WIN_ELEMS^2) * 16^-0.5, folded into the exp() scale
SCALE_TOTAL = float(16.0 ** -0.5 / (WIN_ELEMS * WIN_ELEMS))

F32 = mybir.dt.float32
BF16 = mybir.dt.bfloat16
NP_BF16 = ml_dtypes.bfloat16
AF = mybir.ActivationFunctionType
# CoreSim does not implement Gelu; tests may swap this for AF.Copy.
GELU_FUNC = AF.Gelu


def build_kernel(ctx: ExitStack, tc: "tile.TileContext", yt: bass.AP,
                 xt: bass.AP, ident: bass.AP, zeros: bass.AP,
                 repeats: int = 1):
    """Emit the per-core program.

    xt:    [NB, 128, HWH] bf16 input  (128 = g*64+c)
    ident: [128, 128] f32 identity (for PE transposes)
    zeros: [128, 128] bf16 zeros (lhsT2 off-diagonal init)
    yt:    [NB, 128, HWH] bf16 output (128 = g*64+t)
    """
    nc = tc.nc
    assert CH % W == 0
    rows_per_chunk = CH // W          # h rows per chunk (within a half)
    half_rows = H // G                # 128 rows per half
    gh_per_half = half_rows // WIN_H  # 2

    const_pool = ctx.enter_context(tc.tile_pool(name="const", bufs=1))
    x_pool = ctx.enter_context(tc.tile_pool(name="x", bufs=NB * N_CHUNKS + 1))
    o_pool = ctx.enter_context(tc.tile_pool(name="o", bufs=2))
    r_pool = ctx.enter_context(tc.tile_pool(name="r", bufs=2))
    sm_pool = ctx.enter_context(tc.tile_pool(name="sm", bufs=2))
    # Single PSUM pool: 2 rotating 4-bank slots cover the whole 8-bank PSUM.
    # The tiny softmax-path tiles share the same tag (and thus slots); their
    # stalls against in-flight gelu drains are a few us and overlap the other
    # batch's work.
    ps_pool = ctx.enter_context(tc.tile_pool(name="ps", bufs=2, space="PSUM"))

    ident_sb = const_pool.tile([128, 128], F32)
    nc.sync.dma_start(ident_sb[:], ident)
    # Per-batch-slot block-diag stationary matrices, zeroed once; only the
    # two diagonal 64x64 blocks are rewritten each batch.
    lhsT2s = []
    for b in range(NB):
        t = const_pool.tile([128, 128], BF16, tag=f"lhsT2_{b}")
        nc.sync.dma_start(t[:], zeros)
        lhsT2s.append(t)

    for b in [b for _ in range(repeats) for b in range(NB)]:
        # ---- Phase 1: load chunks, accumulate row-window partial sums ----
        # R1[p, h*4 + gw] = sum_w x[b, hw(g, h, gw, w), c]   (h local to half)
        r1 = r_pool.tile([128, half_rows * 4], F32, tag="r1")
        chunks = []
        for k in range(N_CHUNKS):
            xc = x_pool.tile([128, CH], BF16, tag="xc")
            nc.sync.dma_start(xc[:], xt[b, :, k * CH:(k + 1) * CH])
            chunks.append(xc)
            nc.vector.reduce_sum(
                r1[:, k * rows_per_chunk * 4:(k + 1) * rows_per_chunk * 4],
                xc[:].rearrange("p (h g w) -> p h g w",
                                h=rows_per_chunk, g=4, w=WIN_W),
                axis=mybir.AxisListType.X,
            )

        # ---- Phase 2: finish pooling, attention matrix ----
        # pooled[p, gh*4+gw] = sum over the window (gh local to half)
        pooled = sm_pool.tile([128, 8], F32, tag="pooled")
        nc.vector.reduce_sum(
            pooled[:],
            r1[:].rearrange("p (gh h g) -> p gh g h", gh=gh_per_half,
                            h=WIN_H, g=4),
            axis=mybir.AxisListType.X,
        )
        # pooledT: [8, 128] = pooled.T via PE transpose
        pt_ps = ps_pool.tile([8, 128], F32, tag="ps", name="pt_ps")
        nc.tensor.transpose(pt_ps[:], pooled[:], ident_sb[:])
        p8 = sm_pool.tile([8, 128], F32, tag="p8")
        nc.vector.tensor_copy(p8[:], pt_ps[:])

        # dots[c1, c2] = sum over all 16 windows (8 per half, summed in PSUM)
        dots_ps = ps_pool.tile([64, 64], F32, tag="ps", name="dots_ps")
        nc.tensor.matmul(dots_ps[:], p8[:, 0:64], p8[:, 0:64],
                         start=True, stop=False)
        nc.tensor.matmul(dots_ps[:], p8[:, 64:128], p8[:, 64:128],
                         start=False, stop=True)

        # softmax over rows: attn = exp(s*dots - max)/sum
        mx = sm_pool.tile([64, 1], F32, tag="mx")
        nc.vector.reduce_max(mx[:], dots_ps[:], axis=mybir.AxisListType.X)
        nbias = sm_pool.tile([64, 1], F32, tag="nbias")
        nc.vector.tensor_scalar_mul(nbias[:], mx[:], -SCALE_TOTAL)
        e_sb = sm_pool.tile([64, 64], F32, tag="e_sb")
        ssum = sm_pool.tile([64, 1], F32, tag="ssum")
        nc.scalar.activation(e_sb[:], dots_ps[:], AF.Exp, bias=nbias[:],
                             scale=SCALE_TOTAL, accum_out=ssum[:])
        rcp = sm_pool.tile([64, 1], F32, tag="rcp")
        nc.vector.reciprocal(rcp[:], ssum[:])
        attn = sm_pool.tile([64, 64], F32, tag="attn")
        nc.vector.tensor_scalar_mul(attn[:], e_sb[:], rcp[:])

        # lhsT2 = diag(attnT, attnT) [128, 128] bf16; attnT[c,t] = attn[t,c].
        # The g=1 diagonal block is a partition-shifting SBUF->SBUF DMA copy.
        at_ps = ps_pool.tile([64, 64], F32, tag="ps", name="at_ps")
        nc.tensor.transpose(at_ps[:], attn[:], ident_sb[0:64, 0:64])
        lhsT2 = lhsT2s[b]
        nc.vector.tensor_copy(lhsT2[0:64, 0:64], at_ps[:])
        nc.sync.dma_start(lhsT2[64:128, 64:128], lhsT2[0:64, 0:64])

        # ---- Phase 3: main matmul + gelu + store ----
        # One block-diag matmul per PSUM bank: half g uses array rows/cols
        # [64g, 64g+64), contracting c over partitions 64g..64g+63.  Four
        # banks fill one [128, PS_N] tile, drained by a single gelu ACT.
        for k in range(N_CHUNKS):
            xc = chunks[k]
            for s in range(CH // OSTAGE):
                ostage = o_pool.tile([128, OSTAGE], BF16, tag="ostage")
                for t in range(OSTAGE // PS_N):
                    ps = ps_pool.tile([128, PS_N], F32, tag="ps", name="ps")
                    for j in range(PS_N // MM_N):
                        col = s * OSTAGE + t * PS_N + j * MM_N
                        nc.tensor.matmul(ps[:, j * MM_N:(j + 1) * MM_N],
                                         lhsT2[:], xc[:, col:col + MM_N],
                                         start=True, stop=True)
                    nc.scalar.activation(
                        ostage[:, t * PS_N:(t + 1) * PS_N], ps[:], GELU_FUNC)
                nc.scalar.dma_start(
                    yt[b, :, k * CH + s * OSTAGE: k * CH + (s + 1) * OSTAGE],
                    ostage[:])


def build_nc(trn_type: str = "TRN2", repeats: int = 1) -> bass.Bass:
    nc = bacc.Bacc(trn_type, debug=False, target_bir_lowering=False)
    xt = nc.dram_tensor("xt", [NB, 128, HWH], BF16, kind="ExternalInput")
    ident = nc.dram_tensor("ident", [128, 128], F32, kind="ExternalInput")
    zeros = nc.dram_tensor("zeros", [128, 128], BF16, kind="ExternalInput")
    yt = nc.dram_tensor("yt", [NB, 128, HWH], BF16, kind="ExternalOutput")
    with tile.TileContext(nc) as tc:
        with ExitStack() as ctx:
            build_kernel(ctx, tc, yt.ap(), xt.ap(), ident.ap(), zeros.ap(),
                         repeats=repeats)
    nc.compile()
    return nc


def _pack_inputs(x: np.ndarray) -> np.ndarray:
    # x [B, H, W, C] -> xt [B, 128, HWH]; xt[b, g*64+c, f] = x[b, g*HWH+f, c]
    xr = x.reshape(B, G, HWH, C).transpose(0, 1, 3, 2)   # [B, G, C, HWH]
    return np.ascontiguousarray(
        xr.reshape(B, G * C, HWH).astype(NP_BF16))


def _unpack_outputs(yt: np.ndarray) -> np.ndarray:
    # yt [B, 128, HWH] bf16 -> y [B, H, W, C] f32
    yr = yt.astype(np.float32).reshape(B, G, C, HWH).transpose(0, 1, 3, 2)
    return np.ascontiguousarray(yr.reshape(B, H, W, C))


def _ident_np() -> np.ndarray:
    return np.eye(128, dtype=np.float32)


def _zeros_np() -> np.ndarray:
    return np.zeros((128, 128), dtype=NP_BF16)


_cached = {}


def kernel(x: np.ndarray) -> np.ndarray:
    x = np.asarray(x, dtype=np.float32)
    assert x.shape == (B, H, W, C)
    xt = _pack_inputs(x)

    if "nc" not in _cached:
        _cached["nc"] = build_nc()
    nc = _cached["nc"]

    in_maps = [
        {"xt": np.ascontiguousarray(xt[i * NB:(i + 1) * NB]),
         "ident": _ident_np(), "zeros": _zeros_np()}
        for i in range(N_CORES)
    ]
    res = run_bass_kernel_spmd(nc, in_maps, core_ids=list(range(N_CORES)))
    yt = np.concatenate([r["yt"] for r in res.results], axis=0)
    return _unpack_outputs(yt)


if __name__ == "__main__":
    xs = np.random.default_rng(0).standard_normal((B, H, W, C), dtype=np.float32)
    y = kernel(xs)
    print("ok", y.shape, y.dtype)
